# revision 23
# baseline (speedup 1.0000x reference)
"""Trainium2 (8 NeuronCores) kernel for nn_AngularBasis: real spherical
harmonics up to l=8 for N=2,000,000 points.

Strategy: embarrassingly data-parallel over N (shard across 8 cores).
On-device compute is fp16 tensor-tensor products on VectorE (2x perf mode)
with the precision-critical stage-A (1/r, u,v,w, squares) in f32 and all
static constant scaling folded out to a host-side per-column rescale.
Outputs are DMA'd column-wise as fp16 and upconverted/assembled on host.
"""
import sys
import os
import math

sys.path.insert(0, "/opt/trn_rl_repo")

import numpy as np
import concourse.bass as bass
import concourse.mybir as mybir
import concourse.tile as tile
from concourse.alu_op_type import AluOpType
import types as _types

if "antenv.axon_hooks" not in sys.modules:
    try:
        import antenv  # noqa: F401
        import antenv.axon_hooks  # noqa: F401
    except ImportError:
        _m = _types.ModuleType("antenv.axon_hooks")
        _hook_cell = [None]
        _m.set_axon_ntff_profile_hook = lambda h: _hook_cell.__setitem__(0, h)
        _m.get_axon_ntff_profile_hook = lambda: _hook_cell[0]
        sys.modules["antenv.axon_hooks"] = _m
        try:
            import antenv as _antenv
            _antenv.axon_hooks = _m
        except ImportError:
            pass

from concourse.bass_utils import run_bass_kernel_spmd

L_MAX = 8
N = 2_000_000
NCORES = 8
P = 128
FD = 978                  # free-dim elements per tile
NT = 2                    # tiles per core
LANE = FD * NT            # 1956 elements per SBUF partition lane
NC_ELEMS = P * LANE       # 250368 elements per core
NPAD = NCORES * NC_ELEMS  # 2002944 padded global N

F32 = mybir.dt.float32
F16 = mybir.dt.float16
MULT = AluOpType.mult
SUB = AluOpType.subtract
ADD = AluOpType.add
SQRT3 = math.sqrt(3.0)

# ---------------------------------------------------------------------------
# static constants (pure python/numpy, float64)
# ---------------------------------------------------------------------------


def _prefactors(l):
    m = np.arange(-l, l + 1)
    am = np.abs(m)
    fr = np.array(
        [math.factorial(l - a) / math.factorial(l + a) for a in am], dtype=np.float64
    )
    return ((-1.0) ** m) * math.sqrt(2.0) * np.sqrt((2 * l + 1) / (4.0 * math.pi) * fr)


def _qdiag(m):
    v = 1.0
    for k in range(1, m + 1):
        v = -(2 * k - 1) * v
    return v


def _alp_polys():
    import numpy.polynomial.polynomial as Pp

    q = {}
    for m in range(L_MAX + 1):
        q[(m, m)] = np.array([_qdiag(m)])
    for m in range(L_MAX):
        q[(m + 1, m)] = np.array([0.0, (2 * m + 1) * _qdiag(m)])
    for m in range(L_MAX - 1):
        for l in range(m + 2, L_MAX + 1):
            a = (2 * l - 1) / (l - m)
            b = (l + m - 1) / (l - m)
            prev = Pp.polymulx(q[(l - 1, m)])
            q2 = q[(l - 2, m)]
            n = max(len(prev), len(q2))
            q[(l, m)] = a * np.pad(prev, (0, n - len(prev))) - b * np.pad(
                q2, (0, n - len(q2))
            )
    return q


POLYS = _alp_polys()
_wgrid = np.linspace(-1, 1, 4001)
import numpy.polynomial.polynomial as _Pp

MAXABS = {k: np.abs(_Pp.polyval(_wgrid, v)).max() for k, v in POLYS.items()}


def _plan_scales():
    alpha = {}
    for m in range(0, L_MAX - 1):
        for l in (m + 2, m + 3, m + 4):
            if l <= L_MAX:
                alpha[(l, m)] = 1.0 / MAXABS[(l, m)]
        for l in range(m + 5, L_MAX + 1):
            a = (2 * l - 1) / (l - m)
            alpha[(l, m)] = alpha[(l - 1, m)] / a
    return alpha


ALPHA = _plan_scales()

# trig stored-scale bookkeeping (value_stored = lam * true_value)
LAM_C = {m: 1.0 for m in range(1, 9)}
LAM_S = {m: 1.0 for m in range(1, 9)}
LAM_S[6] = 0.5
LAM_S[8] = 0.5

# output slot plan ---------------------------------------------------------
# outp [18, NT, P, 2*FD] fp16:
#   slots 0..7  = trig pairs m=1..8 (half0 = c_m = (m,+m); half1 = s_m)
#   slots 8..14 = w-pairs (m+1, m) for m=1..7
#   slots 15..17 = leftover chain pairs (8,2), (8,4), (8,6)
# outq [9, NT, P, 4*FD] fp16: quads (m, a) covering cols
#   (a,+m), (a,-m), (a+1,+m), (a+1,-m) in quarters 0..3
# outs [8, NT, P, FD] fp16: slot 0 = w16 = (1,0); slots 1..7 = (l,0) l=2..8
W_PAIRS = [(m + 1, m) for m in range(1, 8)]
LEFT_PAIRS = [(8, 2), (8, 4), (8, 6)]
QUADS = []
for _m in range(1, 6):
    _a = _m + 2
    while _a + 1 <= L_MAX:
        QUADS.append((_m, _a))
        _a += 2
assert len(QUADS) == 9, QUADS

# host-side column map: (l, ms) -> ("p"|"s", slot, half_or_None, delta)


def _column_map():
    cols = {}
    for l in range(1, L_MAX + 1):
        pf = _prefactors(l)
        for ms in range(-l, l + 1):
            am = abs(ms)
            pfv = float(pf[ms + l])
            if ms == 0:
                if l == 1:
                    cols[(l, ms)] = ("s", 0, None, pfv / math.sqrt(2.0))
                else:
                    cols[(l, ms)] = (
                        "s", l - 1, None, pfv / math.sqrt(2.0) / ALPHA[(l, 0)]
                    )
            elif am == l:
                half = 0 if ms > 0 else 1
                lam = LAM_C[am] if ms > 0 else LAM_S[am]
                cols[(l, ms)] = ("p", am - 1, half, pfv * _qdiag(l) / lam)
            else:
                half = 0 if ms > 0 else 1
                lam = LAM_C[am] if ms > 0 else LAM_S[am]
                if l == am + 1:
                    slot = 8 + (am - 1)
                    delta = pfv * (2 * am + 1) * _qdiag(am) / lam
                    cols[(l, ms)] = ("p", slot, half, delta)
                elif (l, am) in LEFT_PAIRS:
                    slot = 15 + LEFT_PAIRS.index((l, am))
                    delta = pfv / (ALPHA[(l, am)] * lam)
                    cols[(l, ms)] = ("p", slot, half, delta)
                else:
                    qi = next(
                        i for i, (qm, qa) in enumerate(QUADS)
                        if qm == am and l in (qa, qa + 1)
                    )
                    qa = QUADS[qi][1]
                    quarter = (0 if l == qa else 2) + half
                    delta = pfv / (ALPHA[(l, am)] * lam)
                    cols[(l, ms)] = ("q", qi, quarter, delta)
    return cols


COLMAP = _column_map()

# ---------------------------------------------------------------------------
# walrus workaround: split multi-sync-wait instructions
# ---------------------------------------------------------------------------
_ws_counter = [0]


def _split_sync_waits(nc, max_waits=1):
    for f in nc.m.functions:
        for blk in f.blocks:
            insts = blk.instructions
            new_list = []
            changed = False
            for inst in insts:
                si = inst.sync_info
                waits = list(si.on_wait) if si is not None else []
                if len(waits) > max_waits:
                    keep = waits[:max_waits]
                    extra = waits[max_waits:]
                    for j in range(0, len(extra), max_waits):
                        chunk = extra[j : j + max_waits]
                        nop = mybir.InstNoOp(
                            name=f"waitsplit_{_ws_counter[0]}", engine=inst.engine
                        )
                        _ws_counter[0] += 1
                        nop.sync_info = mybir.SyncInfo(on_wait=chunk, on_update=[])
                        new_list.append(nop)
                    inst.sync_info = mybir.SyncInfo(
                        on_wait=keep, on_update=list(si.on_update)
                    )
                    changed = True
                new_list.append(inst)
            if changed:
                insts[:] = new_list


# ---------------------------------------------------------------------------
# device program
# ---------------------------------------------------------------------------


def _emit_tile(nc, tc, pools, views, t):
    fpool, sqpool, trigpools, spool, qpool, tpool, ppool, w2pool, qqpool = pools
    xv, yv, zv, rv, outp, outs, outq = views
    Sq = mybir.ActivationFunctionType.Square
    Cp = mybir.ActivationFunctionType.Copy

    Ln = mybir.ActivationFunctionType.Ln
    Exp = mybir.ActivationFunctionType.Exp

    def se_affine(out_ap, in_ap, scale, bias):
        nc.scalar.activation(out_ap, in_ap, Cp, bias=float(bias), scale=float(scale))

    def bcast2(ap):
        # [P, FD] -> [P, 2, FD] with stride-0 middle dim (read twice)
        return ap.rearrange("p (o f) -> p o f", o=1).broadcast_to((P, 2, FD))

    # ---- stage A (f32); 1/r = exp(-ln r) on ScalarE (~5e-5 rel) ----
    rt = fpool.tile([P, FD], F32, tag="f32t", name=f"r_{t}")
    nc.sync.dma_start(rt[:], rv[t])
    lnr = fpool.tile([P, FD], F32, tag="f32t", name=f"lnr_{t}")
    nc.scalar.activation(lnr[:], rt[:], Ln, bias=0.0, scale=1.0)
    invr = fpool.tile([P, FD], F32, tag="f32t", name=f"invr_{t}")
    nc.scalar.activation(invr[:], lnr[:], Exp, bias=0.0, scale=-1.0)

    xt = fpool.tile([P, FD], F32, tag="f32t", name=f"x_{t}")
    nc.sync.dma_start(xt[:], xv[t])
    u = fpool.tile([P, FD], F32, tag="f32t", name=f"u_{t}")
    nc.vector.tensor_tensor(u[:], xt[:], invr[:], MULT)
    yt = fpool.tile([P, FD], F32, tag="f32t", name=f"y_{t}")
    nc.sync.dma_start(yt[:], yv[t])
    v = fpool.tile([P, FD], F32, tag="f32t", name=f"v_{t}")
    nc.vector.tensor_tensor(v[:], yt[:], invr[:], MULT)
    zt = fpool.tile([P, FD], F32, tag="f32t", name=f"z_{t}")
    nc.sync.dma_start(zt[:], zv[t])
    w = fpool.tile([P, FD], F32, tag="f32t", name=f"w_{t}")
    nc.vector.tensor_tensor(w[:], zt[:], invr[:], MULT)

    # fp16 copies (ScalarE: DVE is the bottleneck engine)
    trig = {}
    trig[1] = trigpools[1].tile([P, 2 * FD], F16, tag="trig1", name=f"trig1_{t}")
    se_affine(trig[1][:, :FD], u[:], 1.0, 0.0)
    se_affine(trig[1][:, FD:], v[:], 1.0, 0.0)
    w16 = spool.tile([P, FD], F16, tag="w16", name=f"w16_{t}", bufs=2)
    se_affine(w16[:], w[:], 1.0, 0.0)

    # SE squares (fp16 out: feeds 2x-mode DVE subs; validated no accuracy loss)
    SqU = sqpool.tile([P, FD], F16, tag="sqt", name=f"SqU_{t}")
    nc.scalar.activation(SqU[:], u[:], Sq, bias=0.0, scale=1.0)
    SqV = sqpool.tile([P, FD], F16, tag="sqt", name=f"SqV_{t}")
    nc.scalar.activation(SqV[:], v[:], Sq, bias=0.0, scale=1.0)
    U23 = sqpool.tile([P, FD], F16, tag="sqt", name=f"U23_{t}")
    nc.scalar.activation(U23[:], u[:], Sq, bias=0.0, scale=SQRT3)
    V23 = sqpool.tile([P, FD], F16, tag="sqt", name=f"V23_{t}")
    nc.scalar.activation(V23[:], v[:], Sq, bias=0.0, scale=SQRT3)
    W2 = w2pool.tile([P, FD], F32, tag="W2", name=f"W2_{t}")
    nc.scalar.activation(W2[:], w[:], Sq, bias=0.0, scale=1.0)
    W216 = w2pool.tile([P, FD], F16, tag="W216", name=f"W216_{t}")
    se_affine(W216[:], W2[:], 1.0, 0.0)

    def tmp16(nm):
        return tpool.tile([P, FD], F16, tag="t16", name=f"{nm}_{t}")

    # ---- trig chain (fp16) ----
    for m in range(2, 9):
        trig[m] = trigpools[m].tile(
            [P, 2 * FD], F16, tag=f"trig{m}", name=f"trig{m}_{t}"
        )
    c = {m: trig[m][:, :FD] for m in range(1, 9)}
    s = {m: trig[m][:, FD:] for m in range(1, 9)}

    # m=2
    nc.vector.tensor_tensor(c[2], SqU[:], SqV[:], SUB)
    s2raw = tmp16("s2raw")
    nc.vector.tensor_tensor(s2raw[:], c[1], s[1], MULT)
    se_affine(s[2], s2raw[:], 2.0, 0.0)
    # m=3 (poly)
    D3 = tmp16("D3")
    nc.vector.tensor_tensor(D3[:], SqU[:], V23[:], SUB)
    nc.vector.tensor_tensor(c[3], c[1], D3[:], MULT)
    E3 = tmp16("E3")
    nc.vector.tensor_tensor(E3[:], U23[:], SqV[:], SUB)
    nc.vector.tensor_tensor(s[3], s[1], E3[:], MULT)
    # m=4 (double 2): one merged SE Square over the whole [c2|s2] tile
    Sq2 = sqpool.tile([P, 2 * FD], F16, tag="sqpair", name=f"Sq2_{t}", bufs=2)
    nc.scalar.activation(Sq2[:], trig[2][:], Sq, bias=0.0, scale=1.0)
    nc.vector.tensor_tensor(c[4], Sq2[:, :FD], Sq2[:, FD:], SUB)
    s4raw = tmp16("s4raw")
    nc.vector.tensor_tensor(s4raw[:], c[2], s[2], MULT)
    se_affine(s[4], s4raw[:], 2.0, 0.0)
    # m=5 (add 4+1)
    A5 = tmp16("A5")
    nc.vector.tensor_tensor(A5[:], c[4], c[1], MULT)
    B5 = tmp16("B5")
    nc.vector.tensor_tensor(B5[:], s[4], s[1], MULT)
    nc.vector.tensor_tensor(c[5], A5[:], B5[:], SUB)
    C5 = tmp16("C5")
    nc.vector.tensor_tensor(C5[:], s[4], c[1], MULT)
    D5 = tmp16("D5")
    nc.vector.tensor_tensor(D5[:], c[4], s[1], MULT)
    nc.vector.tensor_tensor(s[5], C5[:], D5[:], ADD)
    # m=6 (double 3)
    Sq3 = sqpool.tile([P, 2 * FD], F16, tag="sqpair", name=f"Sq3_{t}", bufs=2)
    nc.scalar.activation(Sq3[:], trig[3][:], Sq, bias=0.0, scale=1.0)
    nc.vector.tensor_tensor(c[6], Sq3[:, :FD], Sq3[:, FD:], SUB)
    nc.vector.tensor_tensor(s[6], c[3], s[3], MULT)  # lam 1/2
    # m=7 (add 3+4)
    A7 = tmp16("A7")
    nc.vector.tensor_tensor(A7[:], c[3], c[4], MULT)
    B7 = tmp16("B7")
    nc.vector.tensor_tensor(B7[:], s[3], s[4], MULT)
    nc.vector.tensor_tensor(c[7], A7[:], B7[:], SUB)
    C7 = tmp16("C7")
    nc.vector.tensor_tensor(C7[:], s[3], c[4], MULT)
    D7 = tmp16("D7")
    nc.vector.tensor_tensor(D7[:], c[3], s[4], MULT)
    nc.vector.tensor_tensor(s[7], C7[:], D7[:], ADD)
    # m=8 (double 4)
    Sq4 = sqpool.tile([P, 2 * FD], F16, tag="sqpair", name=f"Sq4_{t}", bufs=2)
    nc.scalar.activation(Sq4[:], trig[4][:], Sq, bias=0.0, scale=1.0)
    nc.vector.tensor_tensor(c[8], Sq4[:, :FD], Sq4[:, FD:], SUB)
    nc.vector.tensor_tensor(s[8], c[4], s[4], MULT)  # lam 1/2

    # DMA out trig pair tiles + w16
    for m in range(1, 9):
        nc.sync.dma_start(outp[m - 1, t], trig[m][:])
    nc.sync.dma_start(outs[0, t], w16[:])

    # ---- ALP chains + output pairs/quads ----
    qh = {}  # (l, m) -> AP (half of a chain-pair tile, or a lone tile)
    pair_slot = {}
    for i, lm in enumerate(W_PAIRS):
        pair_slot[lm] = 8 + i
    for i, lm in enumerate(LEFT_PAIRS):
        pair_slot[lm] = 15 + i
    quad_idx = {q: i for i, q in enumerate(QUADS)}

    def emit_pair(l, m, qap):
        # one 2*FD-wide TT: [q|q] (stride-0 broadcast) * [c_m|s_m]
        ptile = ppool.tile([P, 2 * FD], F16, tag="pair", name=f"pair_{l}_{m}_{t}")
        ob = ptile[:].rearrange("p (o f) -> p o f", o=2)
        tb = trig[m][:].rearrange("p (o f) -> p o f", o=2)
        nc.vector.tensor_tensor(ob, bcast2(qap), tb, MULT)
        nc.sync.dma_start(outp[pair_slot[(l, m)], t], ptile[:])

    def emit_quad(m, a, gtile):
        # one 4*FD-wide TT: [qa qa qa1 qa1] * [c s c s]
        qt4 = qqpool.tile([P, 4 * FD], F16, tag="quad", name=f"quad_{m}_{a}_{t}")
        ob = qt4[:].rearrange("p (l cs f) -> p l cs f", l=2, cs=2)
        qb = gtile[:].rearrange("p (l o f) -> p l o f", l=2, o=1).broadcast_to(
            (P, 2, 2, FD)
        )
        tb = trig[m][:].rearrange("p (o cs f) -> p o cs f", o=1, cs=2).broadcast_to(
            (P, 2, 2, FD)
        )
        nc.vector.tensor_tensor(ob, qb, tb, MULT)
        nc.sync.dma_start(outq[quad_idx[(m, a)], t], qt4[:])

    # l = m+1 pairs use w16 directly
    for m in range(1, 8):
        emit_pair(m + 1, m, w16[:])

    for m in range(0, L_MAX - 1):
        # allocate storage: m=0 -> lone FD tiles (DMA'd singles);
        # m>=1 -> chain-pair tiles for (m+2,m+3),(m+4,m+5),... plus lone tail
        group_tiles = {}
        if m >= 1:
            a = m + 2
            while a + 1 <= L_MAX:
                gt = qpool.tile(
                    [P, 2 * FD], F16, tag="qt", name=f"qg_{a}_{m}_{t}", bufs=4
                )
                group_tiles[a] = gt
                qh[(a, m)] = gt[:, :FD]
                qh[(a + 1, m)] = gt[:, FD:]
                a += 2
            if a <= L_MAX:  # lone tail l = a
                lt = qpool.tile(
                    [P, FD], F16, tag="qlone", name=f"ql_{a}_{m}_{t}", bufs=2
                )
                qh[(a, m)] = lt[:]

        def qdst(l):
            if m == 0:
                tl = spool.tile([P, FD], F16, tag=f"q{l}0", name=f"q{l}0_{t}")
                qh[(l, m)] = tl[:]
            return qh[(l, m)]

        def done(l):
            # DMA / emit once the value at this l is final
            if m == 0:
                nc.sync.dma_start(outs[l - 1, t], qh[(l, 0)])
            elif l in group_tiles:
                pass  # first half of a quad; emitted when l+1 is done
            elif (l - 1) in group_tiles:
                emit_quad(m, l - 1, group_tiles[l - 1])
            else:
                emit_pair(l, m, qh[(l, m)])

        # l = m+2 : affine of W2 (SE)
        al2 = ALPHA[(m + 2, m)]
        p2 = POLYS[(m + 2, m)]
        se_affine(qdst(m + 2), W2[:], al2 * p2[2], al2 * p2[0])
        done(m + 2)
        # l = m+3
        if m + 3 <= L_MAX:
            l = m + 3
            a_ = (2 * l - 1) / (l - m)
            b_ = (l + m - 1) / (l - m)
            b3p = b_ * (2 * m + 1) * _qdiag(m)
            al3 = ALPHA[(l, m)]
            inner = tmp16(f"in_{l}_{m}")
            se_affine(inner[:], qh[(m + 2, m)], a_ * al3 / al2, -al3 * b3p)
            nc.vector.tensor_tensor(qdst(l), w16[:], inner[:], MULT)
            done(l)
        # l = m+4 : quadratic in W2
        if m + 4 <= L_MAX:
            l = m + 4
            p4 = POLYS[(l, m)]
            al4 = ALPHA[(l, m)]
            h = tmp16(f"h_{l}_{m}")
            se_affine(h[:], W2[:], al4 * p4[4], al4 * p4[2])
            hW = tmp16(f"hW_{l}_{m}")
            nc.vector.tensor_tensor(hW[:], h[:], W216[:], MULT)
            se_affine(qdst(l), hW[:], 1.0, al4 * p4[0])
            done(l)
        # l >= m+5 : generic recurrence
        for l in range(m + 5, L_MAX + 1):
            a_ = (2 * l - 1) / (l - m)
            b_ = (l + m - 1) / (l - m)
            al = ALPHA[(l, m)]
            cc = al * b_ / ALPHA[(l - 2, m)]
            t2 = tmp16(f"t2_{l}_{m}")
            se_affine(t2[:], qh[(l - 2, m)], cc, 0.0)
            t1 = tmp16(f"t1_{l}_{m}")
            nc.vector.tensor_tensor(t1[:], w16[:], qh[(l - 1, m)], MULT)
            nc.vector.tensor_tensor(qdst(l), t1[:], t2[:], SUB)
            done(l)


def build_program():
    nc = bass.Bass()
    xd = nc.dram_tensor("x", [NC_ELEMS], F32, kind="ExternalInput")
    yd = nc.dram_tensor("y", [NC_ELEMS], F32, kind="ExternalInput")
    zd = nc.dram_tensor("z", [NC_ELEMS], F32, kind="ExternalInput")
    rd = nc.dram_tensor("r", [NC_ELEMS], F32, kind="ExternalInput")
    outp = nc.dram_tensor("outp", [18, NT, P, 2 * FD], F16, kind="ExternalOutput")
    outq = nc.dram_tensor("outq", [9, NT, P, 4 * FD], F16, kind="ExternalOutput")
    outs = nc.dram_tensor("outs", [8, NT, P, FD], F16, kind="ExternalOutput")

    xv = xd.rearrange("(p t f) -> t p f", p=P, t=NT)
    yv = yd.rearrange("(p t f) -> t p f", p=P, t=NT)
    zv = zd.rearrange("(p t f) -> t p f", p=P, t=NT)
    rv = rd.rearrange("(p t f) -> t p f", p=P, t=NT)

    with tile.TileContext(nc) as tc:
        import contextlib

        with contextlib.ExitStack() as ctx:
            fpool = ctx.enter_context(tc.tile_pool(name="f32p", bufs=7))
            sqpool = ctx.enter_context(tc.tile_pool(name="sqp", bufs=5))
            trigpools = {
                m: ctx.enter_context(
                    tc.tile_pool(name=f"trigp{m}", bufs=2 if m <= 4 else 1)
                )
                for m in range(1, 9)
            }
            spool = ctx.enter_context(tc.tile_pool(name="sp", bufs=1))
            qpool = ctx.enter_context(tc.tile_pool(name="qp", bufs=6))
            tpool = ctx.enter_context(tc.tile_pool(name="tp", bufs=6))
            ppool = ctx.enter_context(tc.tile_pool(name="pp", bufs=4))
            qqpool = ctx.enter_context(tc.tile_pool(name="qq", bufs=4))
            w2pool = ctx.enter_context(tc.tile_pool(name="w2p", bufs=2))
            pools = (fpool, sqpool, trigpools, spool, qpool, tpool, ppool, w2pool, qqpool)
            views = (xv, yv, zv, rv, outp, outs, outq)
            for t in range(NT):
                _emit_tile(nc, tc, pools, views, t)

    _split_sync_waits(nc)
    return nc


_NC_CACHE = None


def _get_program():
    global _NC_CACHE
    if _NC_CACHE is None:
        _NC_CACHE = build_program()
    return _NC_CACHE


# ---------------------------------------------------------------------------
# host wrapper
# ---------------------------------------------------------------------------


def kernel(x, y, z, r):
    x = np.asarray(x, dtype=np.float32)
    y = np.asarray(y, dtype=np.float32)
    z = np.asarray(z, dtype=np.float32)
    r = np.asarray(r, dtype=np.float32)
    n = x.shape[0]

    def pad(a, fill):
        out = np.full(NPAD, fill, dtype=np.float32)
        out[:n] = a
        return out

    xp, yp, zp, rp = pad(x, 1.0), pad(y, 1.0), pad(z, 1.0), pad(r, 2.0)

    in_maps = []
    for k in range(NCORES):
        sl = slice(k * NC_ELEMS, (k + 1) * NC_ELEMS)
        in_maps.append(
            {"x": xp[sl].copy(), "y": yp[sl].copy(), "z": zp[sl].copy(),
             "r": rp[sl].copy()}
        )

    nc = _get_program()
    trace = bool(os.environ.get("KERNEL_TRACE"))
    res = run_bass_kernel_spmd(nc, in_maps, core_ids=list(range(NCORES)), trace=trace)
    if res.exec_time_ns is not None:
        print(f"HW exec time: {res.exec_time_ns} ns")

    # assemble: column (l, ms) values for core k live in res.results[k]
    # outp: [36, NT, P, 2FD]; outs: [8, NT, P, FD]
    outs_np = [None] * (L_MAX + 1)
    pf0 = float(_prefactors(0)[0])
    outs_np[0] = np.full((n, 1), np.float32(pf0 / math.sqrt(2.0)), dtype=np.float32)

    # pre-extract per-core flattened columns lazily
    results = res.results

    def extract(kind, slot, half):
        chunks = []
        for k in range(NCORES):
            if kind == "p":
                a = results[k]["outp"][slot, :, :, half * FD : (half + 1) * FD]
            elif kind == "q":
                a = results[k]["outq"][slot, :, :, half * FD : (half + 1) * FD]
            else:
                a = results[k]["outs"][slot]
            # a: [NT, P, FD] ; element index n_core = p*NT*FD + t*FD + f
            chunks.append(np.transpose(a, (1, 0, 2)).reshape(-1))
        return np.concatenate(chunks)[:n]

    for l in range(1, L_MAX + 1):
        o = np.empty((n, 2 * l + 1), dtype=np.float32)
        for ms in range(-l, l + 1):
            kind, slot, half, delta = COLMAP[(l, ms)]
            vals = extract(kind, slot, half)
            o[:, ms + l] = vals.astype(np.float32) * np.float32(delta)
        outs_np[l] = o
    return tuple(outs_np)


# revision 24
# speedup vs baseline: 1.0064x; 1.0064x over previous
"""Trainium2 (8 NeuronCores) kernel for nn_AngularBasis: real spherical
harmonics up to l=8 for N=2,000,000 points.

Strategy: embarrassingly data-parallel over N (shard across 8 cores).
On-device compute is fp16 tensor-tensor products on VectorE (2x perf mode)
with the precision-critical stage-A (1/r, u,v,w, squares) in f32 and all
static constant scaling folded out to a host-side per-column rescale.
Outputs are DMA'd column-wise as fp16 and upconverted/assembled on host.
"""
import sys
import os
import math

sys.path.insert(0, "/opt/trn_rl_repo")

import numpy as np
import concourse.bass as bass
import concourse.mybir as mybir
import concourse.tile as tile
from concourse.alu_op_type import AluOpType
import types as _types

if "antenv.axon_hooks" not in sys.modules:
    try:
        import antenv  # noqa: F401
        import antenv.axon_hooks  # noqa: F401
    except ImportError:
        _m = _types.ModuleType("antenv.axon_hooks")
        _hook_cell = [None]
        _m.set_axon_ntff_profile_hook = lambda h: _hook_cell.__setitem__(0, h)
        _m.get_axon_ntff_profile_hook = lambda: _hook_cell[0]
        sys.modules["antenv.axon_hooks"] = _m
        try:
            import antenv as _antenv
            _antenv.axon_hooks = _m
        except ImportError:
            pass

from concourse.bass_utils import run_bass_kernel_spmd

L_MAX = 8
N = 2_000_000
NCORES = 8
P = 128
FD = 978                  # free-dim elements per tile
NT = 2                    # tiles per core
LANE = FD * NT            # 1956 elements per SBUF partition lane
NC_ELEMS = P * LANE       # 250368 elements per core
NPAD = NCORES * NC_ELEMS  # 2002944 padded global N

F32 = mybir.dt.float32
F16 = mybir.dt.float16
MULT = AluOpType.mult
SUB = AluOpType.subtract
ADD = AluOpType.add
SQRT3 = math.sqrt(3.0)

# ---------------------------------------------------------------------------
# static constants (pure python/numpy, float64)
# ---------------------------------------------------------------------------


def _prefactors(l):
    m = np.arange(-l, l + 1)
    am = np.abs(m)
    fr = np.array(
        [math.factorial(l - a) / math.factorial(l + a) for a in am], dtype=np.float64
    )
    return ((-1.0) ** m) * math.sqrt(2.0) * np.sqrt((2 * l + 1) / (4.0 * math.pi) * fr)


def _qdiag(m):
    v = 1.0
    for k in range(1, m + 1):
        v = -(2 * k - 1) * v
    return v


def _alp_polys():
    import numpy.polynomial.polynomial as Pp

    q = {}
    for m in range(L_MAX + 1):
        q[(m, m)] = np.array([_qdiag(m)])
    for m in range(L_MAX):
        q[(m + 1, m)] = np.array([0.0, (2 * m + 1) * _qdiag(m)])
    for m in range(L_MAX - 1):
        for l in range(m + 2, L_MAX + 1):
            a = (2 * l - 1) / (l - m)
            b = (l + m - 1) / (l - m)
            prev = Pp.polymulx(q[(l - 1, m)])
            q2 = q[(l - 2, m)]
            n = max(len(prev), len(q2))
            q[(l, m)] = a * np.pad(prev, (0, n - len(prev))) - b * np.pad(
                q2, (0, n - len(q2))
            )
    return q


POLYS = _alp_polys()
_wgrid = np.linspace(-1, 1, 4001)
import numpy.polynomial.polynomial as _Pp

MAXABS = {k: np.abs(_Pp.polyval(_wgrid, v)).max() for k, v in POLYS.items()}


def _plan_scales():
    alpha = {}
    for m in list(range(0, L_MAX - 1))[::-1]:
        for l in (m + 2, m + 3, m + 4):
            if l <= L_MAX:
                alpha[(l, m)] = 1.0 / MAXABS[(l, m)]
        for l in range(m + 5, L_MAX + 1):
            a = (2 * l - 1) / (l - m)
            alpha[(l, m)] = alpha[(l - 1, m)] / a
    return alpha


ALPHA = _plan_scales()

# trig stored-scale bookkeeping (value_stored = lam * true_value)
LAM_C = {m: 1.0 for m in range(1, 9)}
LAM_S = {m: 1.0 for m in range(1, 9)}
LAM_S[6] = 0.5
LAM_S[8] = 0.5

# output slot plan ---------------------------------------------------------
# outp [18, NT, P, 2*FD] fp16:
#   slots 0..7  = trig pairs m=1..8 (half0 = c_m = (m,+m); half1 = s_m)
#   slots 8..14 = w-pairs (m+1, m) for m=1..7
#   slots 15..17 = leftover chain pairs (8,2), (8,4), (8,6)
# outq [9, NT, P, 4*FD] fp16: quads (m, a) covering cols
#   (a,+m), (a,-m), (a+1,+m), (a+1,-m) in quarters 0..3
# outs [8, NT, P, FD] fp16: slot 0 = w16 = (1,0); slots 1..7 = (l,0) l=2..8
W_PAIRS = [(m + 1, m) for m in range(1, 8)]
LEFT_PAIRS = [(8, 2), (8, 4), (8, 6)]
QUADS = []
for _m in range(1, 6):
    _a = _m + 2
    while _a + 1 <= L_MAX:
        QUADS.append((_m, _a))
        _a += 2
assert len(QUADS) == 9, QUADS

# host-side column map: (l, ms) -> ("p"|"s", slot, half_or_None, delta)


def _column_map():
    cols = {}
    for l in range(1, L_MAX + 1):
        pf = _prefactors(l)
        for ms in range(-l, l + 1):
            am = abs(ms)
            pfv = float(pf[ms + l])
            if ms == 0:
                if l == 1:
                    cols[(l, ms)] = ("s", 0, None, pfv / math.sqrt(2.0))
                else:
                    cols[(l, ms)] = (
                        "s", l - 1, None, pfv / math.sqrt(2.0) / ALPHA[(l, 0)]
                    )
            elif am == l:
                half = 0 if ms > 0 else 1
                lam = LAM_C[am] if ms > 0 else LAM_S[am]
                cols[(l, ms)] = ("p", am - 1, half, pfv * _qdiag(l) / lam)
            else:
                half = 0 if ms > 0 else 1
                lam = LAM_C[am] if ms > 0 else LAM_S[am]
                if l == am + 1:
                    slot = 8 + (am - 1)
                    delta = pfv * (2 * am + 1) * _qdiag(am) / lam
                    cols[(l, ms)] = ("p", slot, half, delta)
                elif (l, am) in LEFT_PAIRS:
                    slot = 15 + LEFT_PAIRS.index((l, am))
                    delta = pfv / (ALPHA[(l, am)] * lam)
                    cols[(l, ms)] = ("p", slot, half, delta)
                else:
                    qi = next(
                        i for i, (qm, qa) in enumerate(QUADS)
                        if qm == am and l in (qa, qa + 1)
                    )
                    qa = QUADS[qi][1]
                    quarter = (0 if l == qa else 2) + half
                    delta = pfv / (ALPHA[(l, am)] * lam)
                    cols[(l, ms)] = ("q", qi, quarter, delta)
    return cols


COLMAP = _column_map()

# ---------------------------------------------------------------------------
# walrus workaround: split multi-sync-wait instructions
# ---------------------------------------------------------------------------
_ws_counter = [0]


def _split_sync_waits(nc, max_waits=1):
    for f in nc.m.functions:
        for blk in f.blocks:
            insts = blk.instructions
            new_list = []
            changed = False
            for inst in insts:
                si = inst.sync_info
                waits = list(si.on_wait) if si is not None else []
                if len(waits) > max_waits:
                    keep = waits[:max_waits]
                    extra = waits[max_waits:]
                    for j in range(0, len(extra), max_waits):
                        chunk = extra[j : j + max_waits]
                        nop = mybir.InstNoOp(
                            name=f"waitsplit_{_ws_counter[0]}", engine=inst.engine
                        )
                        _ws_counter[0] += 1
                        nop.sync_info = mybir.SyncInfo(on_wait=chunk, on_update=[])
                        new_list.append(nop)
                    inst.sync_info = mybir.SyncInfo(
                        on_wait=keep, on_update=list(si.on_update)
                    )
                    changed = True
                new_list.append(inst)
            if changed:
                insts[:] = new_list


# ---------------------------------------------------------------------------
# device program
# ---------------------------------------------------------------------------


def _emit_tile(nc, tc, pools, views, t):
    fpool, sqpool, trigpools, spool, qpool, tpool, ppool, w2pool, qqpool = pools
    xv, yv, zv, rv, outp, outs, outq = views
    Sq = mybir.ActivationFunctionType.Square
    Cp = mybir.ActivationFunctionType.Copy

    Ln = mybir.ActivationFunctionType.Ln
    Exp = mybir.ActivationFunctionType.Exp

    def se_affine(out_ap, in_ap, scale, bias):
        nc.scalar.activation(out_ap, in_ap, Cp, bias=float(bias), scale=float(scale))

    def bcast2(ap):
        # [P, FD] -> [P, 2, FD] with stride-0 middle dim (read twice)
        return ap.rearrange("p (o f) -> p o f", o=1).broadcast_to((P, 2, FD))

    # ---- stage A (f32); 1/r = exp(-ln r) on ScalarE (~5e-5 rel) ----
    rt = fpool.tile([P, FD], F32, tag="f32t", name=f"r_{t}")
    nc.sync.dma_start(rt[:], rv[t])
    lnr = fpool.tile([P, FD], F32, tag="f32t", name=f"lnr_{t}")
    nc.scalar.activation(lnr[:], rt[:], Ln, bias=0.0, scale=1.0)
    invr = fpool.tile([P, FD], F32, tag="f32t", name=f"invr_{t}")
    nc.scalar.activation(invr[:], lnr[:], Exp, bias=0.0, scale=-1.0)

    xt = fpool.tile([P, FD], F32, tag="f32t", name=f"x_{t}")
    nc.sync.dma_start(xt[:], xv[t])
    u = fpool.tile([P, FD], F32, tag="f32t", name=f"u_{t}")
    nc.vector.tensor_tensor(u[:], xt[:], invr[:], MULT)
    yt = fpool.tile([P, FD], F32, tag="f32t", name=f"y_{t}")
    nc.sync.dma_start(yt[:], yv[t])
    v = fpool.tile([P, FD], F32, tag="f32t", name=f"v_{t}")
    nc.vector.tensor_tensor(v[:], yt[:], invr[:], MULT)
    zt = fpool.tile([P, FD], F32, tag="f32t", name=f"z_{t}")
    nc.sync.dma_start(zt[:], zv[t])
    w = fpool.tile([P, FD], F32, tag="f32t", name=f"w_{t}")
    nc.vector.tensor_tensor(w[:], zt[:], invr[:], MULT)

    # fp16 copies (ScalarE: DVE is the bottleneck engine)
    trig = {}
    trig[1] = trigpools[1].tile([P, 2 * FD], F16, tag="trig1", name=f"trig1_{t}")
    se_affine(trig[1][:, :FD], u[:], 1.0, 0.0)
    se_affine(trig[1][:, FD:], v[:], 1.0, 0.0)
    w16 = spool.tile([P, FD], F16, tag="w16", name=f"w16_{t}", bufs=2)
    se_affine(w16[:], w[:], 1.0, 0.0)

    # SE squares (fp16 out: feeds 2x-mode DVE subs; validated no accuracy loss)
    SqU = sqpool.tile([P, FD], F16, tag="sqt", name=f"SqU_{t}")
    nc.scalar.activation(SqU[:], u[:], Sq, bias=0.0, scale=1.0)
    SqV = sqpool.tile([P, FD], F16, tag="sqt", name=f"SqV_{t}")
    nc.scalar.activation(SqV[:], v[:], Sq, bias=0.0, scale=1.0)
    U23 = sqpool.tile([P, FD], F16, tag="sqt", name=f"U23_{t}")
    nc.scalar.activation(U23[:], u[:], Sq, bias=0.0, scale=SQRT3)
    V23 = sqpool.tile([P, FD], F16, tag="sqt", name=f"V23_{t}")
    nc.scalar.activation(V23[:], v[:], Sq, bias=0.0, scale=SQRT3)
    W2 = w2pool.tile([P, FD], F32, tag="W2", name=f"W2_{t}")
    nc.scalar.activation(W2[:], w[:], Sq, bias=0.0, scale=1.0)
    W216 = w2pool.tile([P, FD], F16, tag="W216", name=f"W216_{t}")
    se_affine(W216[:], W2[:], 1.0, 0.0)

    def tmp16(nm):
        return tpool.tile([P, FD], F16, tag="t16", name=f"{nm}_{t}")

    # ---- trig chain (fp16) ----
    for m in range(2, 9):
        trig[m] = trigpools[m].tile(
            [P, 2 * FD], F16, tag=f"trig{m}", name=f"trig{m}_{t}"
        )
    c = {m: trig[m][:, :FD] for m in range(1, 9)}
    s = {m: trig[m][:, FD:] for m in range(1, 9)}

    # m=2
    nc.vector.tensor_tensor(c[2], SqU[:], SqV[:], SUB)
    s2raw = tmp16("s2raw")
    nc.vector.tensor_tensor(s2raw[:], c[1], s[1], MULT)
    se_affine(s[2], s2raw[:], 2.0, 0.0)
    # m=3 (poly)
    D3 = tmp16("D3")
    nc.vector.tensor_tensor(D3[:], SqU[:], V23[:], SUB)
    nc.vector.tensor_tensor(c[3], c[1], D3[:], MULT)
    E3 = tmp16("E3")
    nc.vector.tensor_tensor(E3[:], U23[:], SqV[:], SUB)
    nc.vector.tensor_tensor(s[3], s[1], E3[:], MULT)
    # m=4 (double 2): one merged SE Square over the whole [c2|s2] tile
    Sq2 = sqpool.tile([P, 2 * FD], F16, tag="sqpair", name=f"Sq2_{t}", bufs=2)
    nc.scalar.activation(Sq2[:], trig[2][:], Sq, bias=0.0, scale=1.0)
    nc.vector.tensor_tensor(c[4], Sq2[:, :FD], Sq2[:, FD:], SUB)
    s4raw = tmp16("s4raw")
    nc.vector.tensor_tensor(s4raw[:], c[2], s[2], MULT)
    se_affine(s[4], s4raw[:], 2.0, 0.0)
    # m=5 (add 4+1)
    A5 = tmp16("A5")
    nc.vector.tensor_tensor(A5[:], c[4], c[1], MULT)
    B5 = tmp16("B5")
    nc.vector.tensor_tensor(B5[:], s[4], s[1], MULT)
    nc.vector.tensor_tensor(c[5], A5[:], B5[:], SUB)
    C5 = tmp16("C5")
    nc.vector.tensor_tensor(C5[:], s[4], c[1], MULT)
    D5 = tmp16("D5")
    nc.vector.tensor_tensor(D5[:], c[4], s[1], MULT)
    nc.vector.tensor_tensor(s[5], C5[:], D5[:], ADD)
    # m=6 (double 3)
    Sq3 = sqpool.tile([P, 2 * FD], F16, tag="sqpair", name=f"Sq3_{t}", bufs=2)
    nc.scalar.activation(Sq3[:], trig[3][:], Sq, bias=0.0, scale=1.0)
    nc.vector.tensor_tensor(c[6], Sq3[:, :FD], Sq3[:, FD:], SUB)
    nc.vector.tensor_tensor(s[6], c[3], s[3], MULT)  # lam 1/2
    # m=7 (add 3+4)
    A7 = tmp16("A7")
    nc.vector.tensor_tensor(A7[:], c[3], c[4], MULT)
    B7 = tmp16("B7")
    nc.vector.tensor_tensor(B7[:], s[3], s[4], MULT)
    nc.vector.tensor_tensor(c[7], A7[:], B7[:], SUB)
    C7 = tmp16("C7")
    nc.vector.tensor_tensor(C7[:], s[3], c[4], MULT)
    D7 = tmp16("D7")
    nc.vector.tensor_tensor(D7[:], c[3], s[4], MULT)
    nc.vector.tensor_tensor(s[7], C7[:], D7[:], ADD)
    # m=8 (double 4)
    Sq4 = sqpool.tile([P, 2 * FD], F16, tag="sqpair", name=f"Sq4_{t}", bufs=2)
    nc.scalar.activation(Sq4[:], trig[4][:], Sq, bias=0.0, scale=1.0)
    nc.vector.tensor_tensor(c[8], Sq4[:, :FD], Sq4[:, FD:], SUB)
    nc.vector.tensor_tensor(s[8], c[4], s[4], MULT)  # lam 1/2

    # DMA out trig pair tiles + w16
    for m in range(1, 9):
        nc.sync.dma_start(outp[m - 1, t], trig[m][:])
    nc.sync.dma_start(outs[0, t], w16[:])

    # ---- ALP chains + output pairs/quads ----
    qh = {}  # (l, m) -> AP (half of a chain-pair tile, or a lone tile)
    pair_slot = {}
    for i, lm in enumerate(W_PAIRS):
        pair_slot[lm] = 8 + i
    for i, lm in enumerate(LEFT_PAIRS):
        pair_slot[lm] = 15 + i
    quad_idx = {q: i for i, q in enumerate(QUADS)}

    def emit_pair(l, m, qap):
        # one 2*FD-wide TT: [q|q] (stride-0 broadcast) * [c_m|s_m]
        ptile = ppool.tile([P, 2 * FD], F16, tag="pair", name=f"pair_{l}_{m}_{t}")
        ob = ptile[:].rearrange("p (o f) -> p o f", o=2)
        tb = trig[m][:].rearrange("p (o f) -> p o f", o=2)
        nc.vector.tensor_tensor(ob, bcast2(qap), tb, MULT)
        nc.sync.dma_start(outp[pair_slot[(l, m)], t], ptile[:])

    def emit_quad(m, a, gtile):
        # one 4*FD-wide TT: [qa qa qa1 qa1] * [c s c s]
        qt4 = qqpool.tile([P, 4 * FD], F16, tag="quad", name=f"quad_{m}_{a}_{t}")
        ob = qt4[:].rearrange("p (l cs f) -> p l cs f", l=2, cs=2)
        qb = gtile[:].rearrange("p (l o f) -> p l o f", l=2, o=1).broadcast_to(
            (P, 2, 2, FD)
        )
        tb = trig[m][:].rearrange("p (o cs f) -> p o cs f", o=1, cs=2).broadcast_to(
            (P, 2, 2, FD)
        )
        nc.vector.tensor_tensor(ob, qb, tb, MULT)
        nc.sync.dma_start(outq[quad_idx[(m, a)], t], qt4[:])

    # l = m+1 pairs use w16 directly
    for m in range(1, 8):
        emit_pair(m + 1, m, w16[:])

    for m in list(range(0, L_MAX - 1))[::-1]:
        # allocate storage: m=0 -> lone FD tiles (DMA'd singles);
        # m>=1 -> chain-pair tiles for (m+2,m+3),(m+4,m+5),... plus lone tail
        group_tiles = {}
        if m >= 1:
            a = m + 2
            while a + 1 <= L_MAX:
                gt = qpool.tile(
                    [P, 2 * FD], F16, tag="qt", name=f"qg_{a}_{m}_{t}", bufs=4
                )
                group_tiles[a] = gt
                qh[(a, m)] = gt[:, :FD]
                qh[(a + 1, m)] = gt[:, FD:]
                a += 2
            if a <= L_MAX:  # lone tail l = a
                lt = qpool.tile(
                    [P, FD], F16, tag="qlone", name=f"ql_{a}_{m}_{t}", bufs=2
                )
                qh[(a, m)] = lt[:]

        def qdst(l):
            if m == 0:
                tl = spool.tile([P, FD], F16, tag=f"q{l}0", name=f"q{l}0_{t}")
                qh[(l, m)] = tl[:]
            return qh[(l, m)]

        def done(l):
            # DMA / emit once the value at this l is final
            if m == 0:
                nc.sync.dma_start(outs[l - 1, t], qh[(l, 0)])
            elif l in group_tiles:
                pass  # first half of a quad; emitted when l+1 is done
            elif (l - 1) in group_tiles:
                emit_quad(m, l - 1, group_tiles[l - 1])
            else:
                emit_pair(l, m, qh[(l, m)])

        # l = m+2 : affine of W2 (SE)
        al2 = ALPHA[(m + 2, m)]
        p2 = POLYS[(m + 2, m)]
        se_affine(qdst(m + 2), W2[:], al2 * p2[2], al2 * p2[0])
        done(m + 2)
        # l = m+3
        if m + 3 <= L_MAX:
            l = m + 3
            a_ = (2 * l - 1) / (l - m)
            b_ = (l + m - 1) / (l - m)
            b3p = b_ * (2 * m + 1) * _qdiag(m)
            al3 = ALPHA[(l, m)]
            inner = tmp16(f"in_{l}_{m}")
            se_affine(inner[:], qh[(m + 2, m)], a_ * al3 / al2, -al3 * b3p)
            nc.vector.tensor_tensor(qdst(l), w16[:], inner[:], MULT)
            done(l)
        # l = m+4 : quadratic in W2
        if m + 4 <= L_MAX:
            l = m + 4
            p4 = POLYS[(l, m)]
            al4 = ALPHA[(l, m)]
            h = tmp16(f"h_{l}_{m}")
            se_affine(h[:], W2[:], al4 * p4[4], al4 * p4[2])
            hW = tmp16(f"hW_{l}_{m}")
            nc.vector.tensor_tensor(hW[:], h[:], W216[:], MULT)
            se_affine(qdst(l), hW[:], 1.0, al4 * p4[0])
            done(l)
        # l >= m+5 : generic recurrence
        for l in range(m + 5, L_MAX + 1):
            a_ = (2 * l - 1) / (l - m)
            b_ = (l + m - 1) / (l - m)
            al = ALPHA[(l, m)]
            cc = al * b_ / ALPHA[(l - 2, m)]
            t2 = tmp16(f"t2_{l}_{m}")
            se_affine(t2[:], qh[(l - 2, m)], cc, 0.0)
            t1 = tmp16(f"t1_{l}_{m}")
            nc.vector.tensor_tensor(t1[:], w16[:], qh[(l - 1, m)], MULT)
            nc.vector.tensor_tensor(qdst(l), t1[:], t2[:], SUB)
            done(l)


def build_program():
    nc = bass.Bass()
    xd = nc.dram_tensor("x", [NC_ELEMS], F32, kind="ExternalInput")
    yd = nc.dram_tensor("y", [NC_ELEMS], F32, kind="ExternalInput")
    zd = nc.dram_tensor("z", [NC_ELEMS], F32, kind="ExternalInput")
    rd = nc.dram_tensor("r", [NC_ELEMS], F32, kind="ExternalInput")
    outp = nc.dram_tensor("outp", [18, NT, P, 2 * FD], F16, kind="ExternalOutput")
    outq = nc.dram_tensor("outq", [9, NT, P, 4 * FD], F16, kind="ExternalOutput")
    outs = nc.dram_tensor("outs", [8, NT, P, FD], F16, kind="ExternalOutput")

    xv = xd.rearrange("(p t f) -> t p f", p=P, t=NT)
    yv = yd.rearrange("(p t f) -> t p f", p=P, t=NT)
    zv = zd.rearrange("(p t f) -> t p f", p=P, t=NT)
    rv = rd.rearrange("(p t f) -> t p f", p=P, t=NT)

    with tile.TileContext(nc) as tc:
        import contextlib

        with contextlib.ExitStack() as ctx:
            fpool = ctx.enter_context(tc.tile_pool(name="f32p", bufs=7))
            sqpool = ctx.enter_context(tc.tile_pool(name="sqp", bufs=5))
            trigpools = {
                m: ctx.enter_context(
                    tc.tile_pool(name=f"trigp{m}", bufs=2 if m <= 4 else 1)
                )
                for m in range(1, 9)
            }
            spool = ctx.enter_context(tc.tile_pool(name="sp", bufs=1))
            qpool = ctx.enter_context(tc.tile_pool(name="qp", bufs=6))
            tpool = ctx.enter_context(tc.tile_pool(name="tp", bufs=6))
            ppool = ctx.enter_context(tc.tile_pool(name="pp", bufs=4))
            qqpool = ctx.enter_context(tc.tile_pool(name="qq", bufs=4))
            w2pool = ctx.enter_context(tc.tile_pool(name="w2p", bufs=2))
            pools = (fpool, sqpool, trigpools, spool, qpool, tpool, ppool, w2pool, qqpool)
            views = (xv, yv, zv, rv, outp, outs, outq)
            for t in range(NT):
                _emit_tile(nc, tc, pools, views, t)

    _split_sync_waits(nc)
    return nc


_NC_CACHE = None


def _get_program():
    global _NC_CACHE
    if _NC_CACHE is None:
        _NC_CACHE = build_program()
    return _NC_CACHE


# ---------------------------------------------------------------------------
# host wrapper
# ---------------------------------------------------------------------------


def kernel(x, y, z, r):
    x = np.asarray(x, dtype=np.float32)
    y = np.asarray(y, dtype=np.float32)
    z = np.asarray(z, dtype=np.float32)
    r = np.asarray(r, dtype=np.float32)
    n = x.shape[0]

    def pad(a, fill):
        out = np.full(NPAD, fill, dtype=np.float32)
        out[:n] = a
        return out

    xp, yp, zp, rp = pad(x, 1.0), pad(y, 1.0), pad(z, 1.0), pad(r, 2.0)

    in_maps = []
    for k in range(NCORES):
        sl = slice(k * NC_ELEMS, (k + 1) * NC_ELEMS)
        in_maps.append(
            {"x": xp[sl].copy(), "y": yp[sl].copy(), "z": zp[sl].copy(),
             "r": rp[sl].copy()}
        )

    nc = _get_program()
    trace = bool(os.environ.get("KERNEL_TRACE"))
    res = run_bass_kernel_spmd(nc, in_maps, core_ids=list(range(NCORES)), trace=trace)
    if res.exec_time_ns is not None:
        print(f"HW exec time: {res.exec_time_ns} ns")

    # assemble: column (l, ms) values for core k live in res.results[k]
    # outp: [36, NT, P, 2FD]; outs: [8, NT, P, FD]
    outs_np = [None] * (L_MAX + 1)
    pf0 = float(_prefactors(0)[0])
    outs_np[0] = np.full((n, 1), np.float32(pf0 / math.sqrt(2.0)), dtype=np.float32)

    # pre-extract per-core flattened columns lazily
    results = res.results

    def extract(kind, slot, half):
        chunks = []
        for k in range(NCORES):
            if kind == "p":
                a = results[k]["outp"][slot, :, :, half * FD : (half + 1) * FD]
            elif kind == "q":
                a = results[k]["outq"][slot, :, :, half * FD : (half + 1) * FD]
            else:
                a = results[k]["outs"][slot]
            # a: [NT, P, FD] ; element index n_core = p*NT*FD + t*FD + f
            chunks.append(np.transpose(a, (1, 0, 2)).reshape(-1))
        return np.concatenate(chunks)[:n]

    for l in range(1, L_MAX + 1):
        o = np.empty((n, 2 * l + 1), dtype=np.float32)
        for ms in range(-l, l + 1):
            kind, slot, half, delta = COLMAP[(l, ms)]
            vals = extract(kind, slot, half)
            o[:, ms + l] = vals.astype(np.float32) * np.float32(delta)
        outs_np[l] = o
    return tuple(outs_np)


# revision 25
# speedup vs baseline: 1.1168x; 1.1097x over previous
"""Trainium2 (8 NeuronCores) kernel for nn_AngularBasis: real spherical
harmonics up to l=8 for N=2,000,000 points.

Strategy: embarrassingly data-parallel over N (shard across 8 cores).
On-device compute is fp16 tensor-tensor products on VectorE (2x perf mode)
with the precision-critical stage-A (1/r, u,v,w, squares) in f32 and all
static constant scaling folded out to a host-side per-column rescale.
Outputs are DMA'd column-wise as fp16 and upconverted/assembled on host.
"""
import sys
import os
import math

sys.path.insert(0, "/opt/trn_rl_repo")

import numpy as np
import concourse.bass as bass
import concourse.mybir as mybir
import concourse.tile as tile
from concourse.alu_op_type import AluOpType
import types as _types

if "antenv.axon_hooks" not in sys.modules:
    try:
        import antenv  # noqa: F401
        import antenv.axon_hooks  # noqa: F401
    except ImportError:
        _m = _types.ModuleType("antenv.axon_hooks")
        _hook_cell = [None]
        _m.set_axon_ntff_profile_hook = lambda h: _hook_cell.__setitem__(0, h)
        _m.get_axon_ntff_profile_hook = lambda: _hook_cell[0]
        sys.modules["antenv.axon_hooks"] = _m
        try:
            import antenv as _antenv
            _antenv.axon_hooks = _m
        except ImportError:
            pass

from concourse.bass_utils import run_bass_kernel_spmd

L_MAX = 8
N = 2_000_000
NCORES = 8
P = 128
FD = 978                  # free-dim elements per tile
NT = 2                    # tiles per core
LANE = FD * NT            # 1956 elements per SBUF partition lane
NC_ELEMS = P * LANE       # 250368 elements per core
NPAD = NCORES * NC_ELEMS  # 2002944 padded global N

F32 = mybir.dt.float32
F16 = mybir.dt.float16
MULT = AluOpType.mult
SUB = AluOpType.subtract
ADD = AluOpType.add
SQRT3 = math.sqrt(3.0)

# ---------------------------------------------------------------------------
# static constants (pure python/numpy, float64)
# ---------------------------------------------------------------------------


def _prefactors(l):
    m = np.arange(-l, l + 1)
    am = np.abs(m)
    fr = np.array(
        [math.factorial(l - a) / math.factorial(l + a) for a in am], dtype=np.float64
    )
    return ((-1.0) ** m) * math.sqrt(2.0) * np.sqrt((2 * l + 1) / (4.0 * math.pi) * fr)


def _qdiag(m):
    v = 1.0
    for k in range(1, m + 1):
        v = -(2 * k - 1) * v
    return v


def _alp_polys():
    import numpy.polynomial.polynomial as Pp

    q = {}
    for m in range(L_MAX + 1):
        q[(m, m)] = np.array([_qdiag(m)])
    for m in range(L_MAX):
        q[(m + 1, m)] = np.array([0.0, (2 * m + 1) * _qdiag(m)])
    for m in range(L_MAX - 1):
        for l in range(m + 2, L_MAX + 1):
            a = (2 * l - 1) / (l - m)
            b = (l + m - 1) / (l - m)
            prev = Pp.polymulx(q[(l - 1, m)])
            q2 = q[(l - 2, m)]
            n = max(len(prev), len(q2))
            q[(l, m)] = a * np.pad(prev, (0, n - len(prev))) - b * np.pad(
                q2, (0, n - len(q2))
            )
    return q


POLYS = _alp_polys()
_wgrid = np.linspace(-1, 1, 4001)
import numpy.polynomial.polynomial as _Pp

MAXABS = {k: np.abs(_Pp.polyval(_wgrid, v)).max() for k, v in POLYS.items()}


def _plan_scales():
    alpha = {}
    for m in range(0, L_MAX - 1):
        for l in (m + 2, m + 3, m + 4):
            if l <= L_MAX:
                alpha[(l, m)] = 1.0 / MAXABS[(l, m)]
        for l in range(m + 5, L_MAX + 1):
            a = (2 * l - 1) / (l - m)
            alpha[(l, m)] = alpha[(l - 1, m)] / a
    return alpha


ALPHA = _plan_scales()

# trig stored-scale bookkeeping (value_stored = lam * true_value)
LAM_C = {m: 1.0 for m in range(1, 9)}
LAM_S = {m: 1.0 for m in range(1, 9)}
LAM_S[6] = 0.5
LAM_S[8] = 0.5

# output slot plan ---------------------------------------------------------
# outp [18, NT, P, 2*FD] fp16:
#   slots 0..7  = trig pairs m=1..8 (half0 = c_m = (m,+m); half1 = s_m)
#   slots 8..14 = w-pairs (m+1, m) for m=1..7
#   slots 15..17 = leftover chain pairs (8,2), (8,4), (8,6)
# outq [9, NT, P, 4*FD] fp16: quads (m, a) covering cols
#   (a,+m), (a,-m), (a+1,+m), (a+1,-m) in quarters 0..3
# outs [8, NT, P, FD] fp16: slot 0 = w16 = (1,0); slots 1..7 = (l,0) l=2..8
W_PAIRS = [(m + 1, m) for m in range(1, 8)]
LEFT_PAIRS = [(8, 2), (8, 4), (8, 6)]
QUADS = []
for _m in range(1, 6):
    _a = _m + 2
    while _a + 1 <= L_MAX:
        QUADS.append((_m, _a))
        _a += 2
assert len(QUADS) == 9, QUADS

# host-side column map: (l, ms) -> ("p"|"s", slot, half_or_None, delta)


def _column_map():
    cols = {}
    for l in range(1, L_MAX + 1):
        pf = _prefactors(l)
        for ms in range(-l, l + 1):
            am = abs(ms)
            pfv = float(pf[ms + l])
            if ms == 0:
                if l == 1:
                    cols[(l, ms)] = ("s", 0, None, pfv / math.sqrt(2.0))
                else:
                    cols[(l, ms)] = (
                        "s", l - 1, None, pfv / math.sqrt(2.0) / ALPHA[(l, 0)]
                    )
            elif am == l:
                half = 0 if ms > 0 else 1
                lam = LAM_C[am] if ms > 0 else LAM_S[am]
                cols[(l, ms)] = ("p", am - 1, half, pfv * _qdiag(l) / lam)
            else:
                half = 0 if ms > 0 else 1
                lam = LAM_C[am] if ms > 0 else LAM_S[am]
                if l == am + 1:
                    slot = 8 + (am - 1)
                    delta = pfv * (2 * am + 1) * _qdiag(am) / lam
                    cols[(l, ms)] = ("p", slot, half, delta)
                elif (l, am) in LEFT_PAIRS:
                    slot = 15 + LEFT_PAIRS.index((l, am))
                    delta = pfv / (ALPHA[(l, am)] * lam)
                    cols[(l, ms)] = ("p", slot, half, delta)
                else:
                    qi = next(
                        i for i, (qm, qa) in enumerate(QUADS)
                        if qm == am and l in (qa, qa + 1)
                    )
                    qa = QUADS[qi][1]
                    quarter = (0 if l == qa else 2) + half
                    delta = pfv / (ALPHA[(l, am)] * lam)
                    cols[(l, ms)] = ("q", qi, quarter, delta)
    return cols


COLMAP = _column_map()

# ---------------------------------------------------------------------------
# walrus workaround: split multi-sync-wait instructions
# ---------------------------------------------------------------------------
_ws_counter = [0]


def _split_sync_waits(nc, max_waits=1):
    for f in nc.m.functions:
        for blk in f.blocks:
            insts = blk.instructions
            new_list = []
            changed = False
            for inst in insts:
                si = inst.sync_info
                waits = list(si.on_wait) if si is not None else []
                if len(waits) > max_waits:
                    keep = waits[:max_waits]
                    extra = waits[max_waits:]
                    for j in range(0, len(extra), max_waits):
                        chunk = extra[j : j + max_waits]
                        nop = mybir.InstNoOp(
                            name=f"waitsplit_{_ws_counter[0]}", engine=inst.engine
                        )
                        _ws_counter[0] += 1
                        nop.sync_info = mybir.SyncInfo(on_wait=chunk, on_update=[])
                        new_list.append(nop)
                    inst.sync_info = mybir.SyncInfo(
                        on_wait=keep, on_update=list(si.on_update)
                    )
                    changed = True
                new_list.append(inst)
            if changed:
                insts[:] = new_list


# ---------------------------------------------------------------------------
# device program
# ---------------------------------------------------------------------------


def _emit_tile(nc, tc, pools, views, t):
    fpool, sqpool, trigpools, spool, qpool, tpool, ppool, w2pool, qqpool = pools
    xv, yv, zv, rv, outp, outs, outq = views
    Sq = mybir.ActivationFunctionType.Square
    Cp = mybir.ActivationFunctionType.Copy

    Ln = mybir.ActivationFunctionType.Ln
    Exp = mybir.ActivationFunctionType.Exp

    def se_affine(out_ap, in_ap, scale, bias):
        nc.scalar.activation(out_ap, in_ap, Cp, bias=float(bias), scale=float(scale))

    def bcast2(ap):
        # [P, FD] -> [P, 2, FD] with stride-0 middle dim (read twice)
        return ap.rearrange("p (o f) -> p o f", o=1).broadcast_to((P, 2, FD))

    # ---- stage A (f32); 1/r = exp(-ln r) on ScalarE (~5e-5 rel) ----
    rt = fpool.tile([P, FD], F32, tag="f32t", name=f"r_{t}")
    nc.sync.dma_start(rt[:], rv[t])
    lnr = fpool.tile([P, FD], F32, tag="f32t", name=f"lnr_{t}")
    nc.scalar.activation(lnr[:], rt[:], Ln, bias=0.0, scale=1.0)
    invr = fpool.tile([P, FD], F32, tag="f32t", name=f"invr_{t}")
    nc.scalar.activation(invr[:], lnr[:], Exp, bias=0.0, scale=-1.0)

    xt = fpool.tile([P, FD], F32, tag="f32t", name=f"x_{t}")
    nc.sync.dma_start(xt[:], xv[t])
    u = fpool.tile([P, FD], F32, tag="f32t", name=f"u_{t}")
    nc.vector.tensor_tensor(u[:], xt[:], invr[:], MULT)
    yt = fpool.tile([P, FD], F32, tag="f32t", name=f"y_{t}")
    nc.sync.dma_start(yt[:], yv[t])
    v = fpool.tile([P, FD], F32, tag="f32t", name=f"v_{t}")
    nc.vector.tensor_tensor(v[:], yt[:], invr[:], MULT)
    zt = fpool.tile([P, FD], F32, tag="f32t", name=f"z_{t}")
    nc.sync.dma_start(zt[:], zv[t])
    w = fpool.tile([P, FD], F32, tag="f32t", name=f"w_{t}")
    nc.vector.tensor_tensor(w[:], zt[:], invr[:], MULT)

    # fp16 copies (ScalarE: DVE is the bottleneck engine)
    trig = {}
    trig[1] = trigpools[1].tile([P, 2 * FD], F16, tag="trig1", name=f"trig1_{t}")
    se_affine(trig[1][:, :FD], u[:], 1.0, 0.0)
    se_affine(trig[1][:, FD:], v[:], 1.0, 0.0)
    w16 = spool.tile([P, FD], F16, tag="w16", name=f"w16_{t}", bufs=2)
    se_affine(w16[:], w[:], 1.0, 0.0)

    # SE squares (fp16 out: feeds 2x-mode DVE subs; validated no accuracy loss)
    SqU = sqpool.tile([P, FD], F16, tag="sqt", name=f"SqU_{t}")
    nc.scalar.activation(SqU[:], u[:], Sq, bias=0.0, scale=1.0)
    SqV = sqpool.tile([P, FD], F16, tag="sqt", name=f"SqV_{t}")
    nc.scalar.activation(SqV[:], v[:], Sq, bias=0.0, scale=1.0)
    U23 = sqpool.tile([P, FD], F16, tag="sqt", name=f"U23_{t}")
    nc.scalar.activation(U23[:], u[:], Sq, bias=0.0, scale=SQRT3)
    V23 = sqpool.tile([P, FD], F16, tag="sqt", name=f"V23_{t}")
    nc.scalar.activation(V23[:], v[:], Sq, bias=0.0, scale=SQRT3)
    W2 = w2pool.tile([P, FD], F32, tag="W2", name=f"W2_{t}")
    nc.scalar.activation(W2[:], w[:], Sq, bias=0.0, scale=1.0)
    W216 = w2pool.tile([P, FD], F16, tag="W216", name=f"W216_{t}")
    se_affine(W216[:], W2[:], 1.0, 0.0)

    def tmp16(nm):
        return tpool.tile([P, FD], F16, tag="t16", name=f"{nm}_{t}")

    # ---- trig chain (fp16) ----
    for m in range(2, 9):
        trig[m] = trigpools[m].tile(
            [P, 2 * FD], F16, tag=f"trig{m}", name=f"trig{m}_{t}"
        )
    c = {m: trig[m][:, :FD] for m in range(1, 9)}
    s = {m: trig[m][:, FD:] for m in range(1, 9)}

    # m=2
    nc.vector.tensor_tensor(c[2], SqU[:], SqV[:], SUB)
    s2raw = tmp16("s2raw")
    nc.vector.tensor_tensor(s2raw[:], c[1], s[1], MULT)
    se_affine(s[2], s2raw[:], 2.0, 0.0)
    # m=3 (poly)
    D3 = tmp16("D3")
    nc.vector.tensor_tensor(D3[:], SqU[:], V23[:], SUB)
    nc.vector.tensor_tensor(c[3], c[1], D3[:], MULT)
    E3 = tmp16("E3")
    nc.vector.tensor_tensor(E3[:], U23[:], SqV[:], SUB)
    nc.vector.tensor_tensor(s[3], s[1], E3[:], MULT)
    # m=4 (double 2): one merged SE Square over the whole [c2|s2] tile
    Sq2 = sqpool.tile([P, 2 * FD], F16, tag="sqpair", name=f"Sq2_{t}", bufs=2)
    nc.scalar.activation(Sq2[:], trig[2][:], Sq, bias=0.0, scale=1.0)
    nc.vector.tensor_tensor(c[4], Sq2[:, :FD], Sq2[:, FD:], SUB)
    s4raw = tmp16("s4raw")
    nc.vector.tensor_tensor(s4raw[:], c[2], s[2], MULT)
    se_affine(s[4], s4raw[:], 2.0, 0.0)
    # m=5 (add 4+1)
    A5 = tmp16("A5")
    nc.vector.tensor_tensor(A5[:], c[4], c[1], MULT)
    B5 = tmp16("B5")
    nc.vector.tensor_tensor(B5[:], s[4], s[1], MULT)
    nc.vector.tensor_tensor(c[5], A5[:], B5[:], SUB)
    C5 = tmp16("C5")
    nc.vector.tensor_tensor(C5[:], s[4], c[1], MULT)
    D5 = tmp16("D5")
    nc.vector.tensor_tensor(D5[:], c[4], s[1], MULT)
    nc.vector.tensor_tensor(s[5], C5[:], D5[:], ADD)
    # m=6 (double 3)
    Sq3 = sqpool.tile([P, 2 * FD], F16, tag="sqpair", name=f"Sq3_{t}", bufs=2)
    nc.scalar.activation(Sq3[:], trig[3][:], Sq, bias=0.0, scale=1.0)
    nc.vector.tensor_tensor(c[6], Sq3[:, :FD], Sq3[:, FD:], SUB)
    nc.vector.tensor_tensor(s[6], c[3], s[3], MULT)  # lam 1/2
    # m=7 (add 3+4)
    A7 = tmp16("A7")
    nc.vector.tensor_tensor(A7[:], c[3], c[4], MULT)
    B7 = tmp16("B7")
    nc.vector.tensor_tensor(B7[:], s[3], s[4], MULT)
    nc.vector.tensor_tensor(c[7], A7[:], B7[:], SUB)
    C7 = tmp16("C7")
    nc.vector.tensor_tensor(C7[:], s[3], c[4], MULT)
    D7 = tmp16("D7")
    nc.vector.tensor_tensor(D7[:], c[3], s[4], MULT)
    nc.vector.tensor_tensor(s[7], C7[:], D7[:], ADD)
    # m=8 (double 4)
    Sq4 = sqpool.tile([P, 2 * FD], F16, tag="sqpair", name=f"Sq4_{t}", bufs=2)
    nc.scalar.activation(Sq4[:], trig[4][:], Sq, bias=0.0, scale=1.0)
    nc.vector.tensor_tensor(c[8], Sq4[:, :FD], Sq4[:, FD:], SUB)
    nc.vector.tensor_tensor(s[8], c[4], s[4], MULT)  # lam 1/2

    # DMA out trig pair tiles + w16
    for m in range(1, 9):
        nc.sync.dma_start(outp[m - 1, t], trig[m][:])
    nc.sync.dma_start(outs[0, t], w16[:])

    # ---- ALP chains + output pairs/quads ----
    qh = {}  # (l, m) -> AP (half of a chain-pair tile, or a lone tile)
    pair_slot = {}
    for i, lm in enumerate(W_PAIRS):
        pair_slot[lm] = 8 + i
    for i, lm in enumerate(LEFT_PAIRS):
        pair_slot[lm] = 15 + i
    quad_idx = {q: i for i, q in enumerate(QUADS)}

    def emit_pair(l, m, qap):
        # one 2*FD-wide TT: [q|q] (stride-0 broadcast) * [c_m|s_m]
        ptile = ppool.tile([P, 2 * FD], F16, tag="pair", name=f"pair_{l}_{m}_{t}")
        ob = ptile[:].rearrange("p (o f) -> p o f", o=2)
        tb = trig[m][:].rearrange("p (o f) -> p o f", o=2)
        nc.vector.tensor_tensor(ob, bcast2(qap), tb, MULT)
        nc.sync.dma_start(outp[pair_slot[(l, m)], t], ptile[:])

    def emit_quad(m, a, gtile):
        # one 4*FD-wide TT: [qa qa qa1 qa1] * [c s c s]
        qt4 = qqpool.tile([P, 4 * FD], F16, tag="quad", name=f"quad_{m}_{a}_{t}")
        ob = qt4[:].rearrange("p (l cs f) -> p l cs f", l=2, cs=2)
        qb = gtile[:].rearrange("p (l o f) -> p l o f", l=2, o=1).broadcast_to(
            (P, 2, 2, FD)
        )
        tb = trig[m][:].rearrange("p (o cs f) -> p o cs f", o=1, cs=2).broadcast_to(
            (P, 2, 2, FD)
        )
        nc.vector.tensor_tensor(ob, qb, tb, MULT)
        nc.sync.dma_start(outq[quad_idx[(m, a)], t], qt4[:])

    # l = m+1 pairs use w16 directly
    for m in range(1, 8):
        emit_pair(m + 1, m, w16[:])

    for m in range(0, L_MAX - 1):
        # allocate storage: m=0 -> lone FD tiles (DMA'd singles);
        # m>=1 -> chain-pair tiles for (m+2,m+3),(m+4,m+5),... plus lone tail
        group_tiles = {}
        if m >= 1:
            a = m + 2
            while a + 1 <= L_MAX:
                gt = qpool.tile(
                    [P, 2 * FD], F16, tag="qt", name=f"qg_{a}_{m}_{t}", bufs=4
                )
                group_tiles[a] = gt
                qh[(a, m)] = gt[:, :FD]
                qh[(a + 1, m)] = gt[:, FD:]
                a += 2
            if a <= L_MAX:  # lone tail l = a
                lt = qpool.tile(
                    [P, FD], F16, tag="qlone", name=f"ql_{a}_{m}_{t}", bufs=2
                )
                qh[(a, m)] = lt[:]

        def qdst(l):
            if m == 0:
                tl = spool.tile([P, FD], F16, tag=f"q{l}0", name=f"q{l}0_{t}")
                qh[(l, m)] = tl[:]
            return qh[(l, m)]

        def done(l):
            # DMA / emit once the value at this l is final
            if m == 0:
                nc.sync.dma_start(outs[l - 1, t], qh[(l, 0)])
            elif l in group_tiles:
                pass  # first half of a quad; emitted when l+1 is done
            elif (l - 1) in group_tiles:
                emit_quad(m, l - 1, group_tiles[l - 1])
            else:
                emit_pair(l, m, qh[(l, m)])

        # l = m+2 : affine of W2 (SE)
        al2 = ALPHA[(m + 2, m)]
        p2 = POLYS[(m + 2, m)]
        se_affine(qdst(m + 2), W2[:], al2 * p2[2], al2 * p2[0])
        done(m + 2)
        # l = m+3
        if m + 3 <= L_MAX:
            l = m + 3
            a_ = (2 * l - 1) / (l - m)
            b_ = (l + m - 1) / (l - m)
            b3p = b_ * (2 * m + 1) * _qdiag(m)
            al3 = ALPHA[(l, m)]
            inner = tmp16(f"in_{l}_{m}")
            se_affine(inner[:], qh[(m + 2, m)], a_ * al3 / al2, -al3 * b3p)
            nc.vector.tensor_tensor(qdst(l), w16[:], inner[:], MULT)
            done(l)
        # l = m+4 : quadratic in W2
        if m + 4 <= L_MAX:
            l = m + 4
            p4 = POLYS[(l, m)]
            al4 = ALPHA[(l, m)]
            h = tmp16(f"h_{l}_{m}")
            se_affine(h[:], W2[:], al4 * p4[4], al4 * p4[2])
            hW = tmp16(f"hW_{l}_{m}")
            nc.vector.tensor_tensor(hW[:], h[:], W216[:], MULT)
            se_affine(qdst(l), hW[:], 1.0, al4 * p4[0])
            done(l)
        # l >= m+5 : generic recurrence
        for l in range(m + 5, L_MAX + 1):
            a_ = (2 * l - 1) / (l - m)
            b_ = (l + m - 1) / (l - m)
            al = ALPHA[(l, m)]
            cc = al * b_ / ALPHA[(l - 2, m)]
            t2 = tmp16(f"t2_{l}_{m}")
            se_affine(t2[:], qh[(l - 2, m)], cc, 0.0)
            t1 = tmp16(f"t1_{l}_{m}")
            nc.vector.tensor_tensor(t1[:], w16[:], qh[(l - 1, m)], MULT)
            nc.vector.tensor_tensor(qdst(l), t1[:], t2[:], SUB)
            done(l)


def build_program():
    nc = bass.Bass()
    xd = nc.dram_tensor("x", [NC_ELEMS], F32, kind="ExternalInput")
    yd = nc.dram_tensor("y", [NC_ELEMS], F32, kind="ExternalInput")
    zd = nc.dram_tensor("z", [NC_ELEMS], F32, kind="ExternalInput")
    rd = nc.dram_tensor("r", [NC_ELEMS], F32, kind="ExternalInput")
    outp = nc.dram_tensor("outp", [18, NT, P, 2 * FD], F16, kind="ExternalOutput")
    outq = nc.dram_tensor("outq", [9, NT, P, 4 * FD], F16, kind="ExternalOutput")
    outs = nc.dram_tensor("outs", [8, NT, P, FD], F16, kind="ExternalOutput")

    xv = xd.rearrange("(p t f) -> t p f", p=P, t=NT)
    yv = yd.rearrange("(p t f) -> t p f", p=P, t=NT)
    zv = zd.rearrange("(p t f) -> t p f", p=P, t=NT)
    rv = rd.rearrange("(p t f) -> t p f", p=P, t=NT)

    with tile.TileContext(nc) as tc:
        import contextlib

        with contextlib.ExitStack() as ctx:
            fpool = ctx.enter_context(tc.tile_pool(name="f32p", bufs=7))
            sqpool = ctx.enter_context(tc.tile_pool(name="sqp", bufs=5))
            trigpools = {
                m: ctx.enter_context(
                    tc.tile_pool(name=f"trigp{m}", bufs=2 if m <= 4 else 1)
                )
                for m in range(1, 9)
            }
            spool = ctx.enter_context(tc.tile_pool(name="sp", bufs=1))
            qpool = ctx.enter_context(tc.tile_pool(name="qp", bufs=6))
            tpool = ctx.enter_context(tc.tile_pool(name="tp", bufs=6))
            ppool = ctx.enter_context(tc.tile_pool(name="pp", bufs=4))
            qqpool = ctx.enter_context(tc.tile_pool(name="qq", bufs=4))
            w2pool = ctx.enter_context(tc.tile_pool(name="w2p", bufs=2))
            pools = (fpool, sqpool, trigpools, spool, qpool, tpool, ppool, w2pool, qqpool)
            views = (xv, yv, zv, rv, outp, outs, outq)
            for t in range(NT):
                _emit_tile(nc, tc, pools, views, t)

    _split_sync_waits(nc)
    return nc


_NC_CACHE = None


def _get_program():
    global _NC_CACHE
    if _NC_CACHE is None:
        _NC_CACHE = build_program()
    return _NC_CACHE


# ---------------------------------------------------------------------------
# host wrapper
# ---------------------------------------------------------------------------


def kernel(x, y, z, r):
    x = np.asarray(x, dtype=np.float32)
    y = np.asarray(y, dtype=np.float32)
    z = np.asarray(z, dtype=np.float32)
    r = np.asarray(r, dtype=np.float32)
    n = x.shape[0]

    def pad(a, fill):
        out = np.full(NPAD, fill, dtype=np.float32)
        out[:n] = a
        return out

    xp, yp, zp, rp = pad(x, 1.0), pad(y, 1.0), pad(z, 1.0), pad(r, 2.0)

    in_maps = []
    for k in range(NCORES):
        sl = slice(k * NC_ELEMS, (k + 1) * NC_ELEMS)
        in_maps.append(
            {"x": xp[sl].copy(), "y": yp[sl].copy(), "z": zp[sl].copy(),
             "r": rp[sl].copy()}
        )

    nc = _get_program()
    trace = bool(os.environ.get("KERNEL_TRACE"))
    res = run_bass_kernel_spmd(nc, in_maps, core_ids=list(range(NCORES)), trace=trace)
    if res.exec_time_ns is not None:
        print(f"HW exec time: {res.exec_time_ns} ns")

    # assemble: column (l, ms) values for core k live in res.results[k]
    # outp: [36, NT, P, 2FD]; outs: [8, NT, P, FD]
    outs_np = [None] * (L_MAX + 1)
    pf0 = float(_prefactors(0)[0])
    outs_np[0] = np.full((n, 1), np.float32(pf0 / math.sqrt(2.0)), dtype=np.float32)

    # pre-extract per-core flattened columns lazily
    results = res.results

    def extract(kind, slot, half):
        chunks = []
        for k in range(NCORES):
            if kind == "p":
                a = results[k]["outp"][slot, :, :, half * FD : (half + 1) * FD]
            elif kind == "q":
                a = results[k]["outq"][slot, :, :, half * FD : (half + 1) * FD]
            else:
                a = results[k]["outs"][slot]
            # a: [NT, P, FD] ; element index n_core = p*NT*FD + t*FD + f
            chunks.append(np.transpose(a, (1, 0, 2)).reshape(-1))
        return np.concatenate(chunks)[:n]

    for l in range(1, L_MAX + 1):
        o = np.empty((n, 2 * l + 1), dtype=np.float32)
        for ms in range(-l, l + 1):
            kind, slot, half, delta = COLMAP[(l, ms)]
            vals = extract(kind, slot, half)
            o[:, ms + l] = vals.astype(np.float32) * np.float32(delta)
        outs_np[l] = o
    return tuple(outs_np)
